# revision 8
# baseline (speedup 1.0000x reference)
"""Trainium2 Bass kernel for GTPCA topk_masking layer.

Computation (see reference):
  wn     = w / sqrt(sum(w^2)/n),  n = 128*128
  scores = valid_xcorr2d(inputs, wn) / n          -> (B, 113, 113)
  idx    = argmax |scores| (flat, first occurrence)
  out    = scores[idx] * wn placed as a 16x16 patch at idx, zeros elsewhere

Device strategy (pure data parallel over 8 cores, 512 images each):
  - The 2D correlation is done on the tensor engine as 16 accumulating
    matmuls per image group: for each kernel column q, a 128x113 Toeplitz
    matrix T_q (T_q[h, i] = wn[h-i, q]) is the stationary operand and the
    width-shifted image slice in[:, j+q] is the moving operand, accumulating
    over q in PSUM.  Uses float32r so the PE streams 1 column/cycle.
  - Per PSUM bank (4 images) a single fused DVE reduce with
    apply_absolute_value produces per-row abs-max of the score map.
  - Only the per-row abs-max [113, 512] leaves the device.  The host finds
    candidate rows within 1% of each image's global max (fp32r noise is
    orders of magnitude smaller), rescores those rows exactly in fp64,
    picks the true argmax, and scatters smax*wn patches into the output.
"""

import sys

import numpy as np

if "/opt/trn_rl_repo" not in sys.path:
    sys.path.insert(0, "/opt/trn_rl_repo")

N_CORES = 8
B = 4096
H = W = 128
KH = KW = 16
SH = SW = H - KH + 1  # 113
SW_PAD = 114  # fp32r matmul needs even innermost moving/psum counts
W_PAD = 130  # padded image width so q+114 stays in bounds, stride even
N_ELEM = H * W  # 16384
PER_CORE = B // N_CORES  # 512
GROUP = 16  # images per DMA/compute group
BANK = 4  # images per PSUM bank (4*113 = 452 <= 512 moving-dim limit)
CAND_TOL = 1e-2  # candidate-row gate vs device global max


def _build_nc(n_imgs: int):
    from contextlib import ExitStack

    import concourse.bacc as bacc
    import concourse.mybir as mybir
    import concourse.tile as tile

    f32 = mybir.dt.float32
    f32r = mybir.dt.float32r

    nc = bacc.Bacc("TRN2", target_bir_lowering=False)
    imgs_d = nc.dram_tensor("imgs", [H, n_imgs, W_PAD], f32r, kind="ExternalInput")
    ttoe_d = nc.dram_tensor("ttoe", [H, KW, SH], f32r, kind="ExternalInput")
    rm_d = nc.dram_tensor("rowmax", [SH, n_imgs], f32, kind="ExternalOutput")

    n_groups = n_imgs // GROUP
    banks_per_group = GROUP // BANK

    with ExitStack() as ctx:
        tc = ctx.enter_context(tile.TileContext(nc))
        consts = ctx.enter_context(tc.tile_pool(name="consts", bufs=1))
        imgp = ctx.enter_context(tc.tile_pool(name="imgp", bufs=3))
        accp = ctx.enter_context(tc.tile_pool(name="accp", bufs=2, space="PSUM"))
        stage = ctx.enter_context(tc.tile_pool(name="stage", bufs=1))

        ttoe_t = consts.tile([H, KW, SH], f32r)
        nc.sync.dma_start(ttoe_t[:], ttoe_d[:])
        rm_all = stage.tile([SH, n_imgs], f32)

        for g in range(n_groups):
            img_t = imgp.tile([H, GROUP, W_PAD], f32r)
            nc.sync.dma_start(img_t[:], imgs_d[:, g * GROUP : (g + 1) * GROUP, :])

            psums = [
                accp.tile([SH, BANK, SW_PAD], f32, name=f"acc{bk}", tag=f"acc{bk}")
                for bk in range(banks_per_group)
            ]
            for q in range(KW):
                lhs = ttoe_t[:, q, :]
                for bk in range(banks_per_group):
                    rhs = img_t[:, bk * BANK : (bk + 1) * BANK, q : q + SW_PAD]
                    nc.tensor.matmul(
                        psums[bk][:],
                        lhs,
                        rhs,
                        start=(q == 0),
                        stop=(q == KW - 1),
                        skip_group_check=True,
                    )
            for bk in range(banks_per_group):
                base = g * GROUP + bk * BANK
                nc.vector.tensor_reduce(
                    rm_all[:, base : base + BANK],
                    psums[bk][:, :, 0:SW],
                    axis=mybir.AxisListType.X,
                    op=mybir.AluOpType.max,
                    apply_absolute_value=True,
                )

        nc.sync.dma_start(rm_d[:], rm_all[:])

    nc.compile()
    return nc


_NC_CACHE: dict = {}


def _get_nc(n_imgs: int):
    if n_imgs not in _NC_CACHE:
        _NC_CACHE[n_imgs] = _build_nc(n_imgs)
    return _NC_CACHE[n_imgs]


def _weights_f32(w: np.ndarray) -> np.ndarray:
    w32 = np.asarray(w, dtype=np.float32)
    ss = np.sum(w32 * w32, dtype=np.float32)
    denom = np.sqrt(ss / np.float32(N_ELEM))
    return (w32 / denom).astype(np.float32)


def _toeplitz(wn: np.ndarray) -> np.ndarray:
    ttoe = np.zeros((H, KW, SH), dtype=np.float32)
    for i in range(SH):
        ttoe[i : i + KH, :, i] = wn
    return ttoe


def _run_device(inputs_np: np.ndarray, ttoe: np.ndarray, trace: bool = False):
    from concourse.bass_utils import run_bass_kernel_spmd

    nc = _get_nc(PER_CORE)
    nb = inputs_np.shape[0]
    host_t = np.zeros((H, nb, W_PAD), dtype=np.float32)
    host_t[:, :, :W] = inputs_np.transpose(1, 0, 2)
    in_maps = []
    for c in range(N_CORES):
        shard = np.ascontiguousarray(
            host_t[:, c * PER_CORE : (c + 1) * PER_CORE, :]
        )
        in_maps.append({"imgs": shard, "ttoe": ttoe})
    res = run_bass_kernel_spmd(
        nc, in_maps, core_ids=list(range(N_CORES)), trace=trace
    )
    rm = np.concatenate([r["rowmax"] for r in res.results], axis=1)  # [113, B]
    return rm, res


def _finalize(inputs_np: np.ndarray, wn: np.ndarray, rm: np.ndarray) -> np.ndarray:
    """Host: candidate rows -> exact rescore -> argmax -> patch scatter."""
    nb = rm.shape[1]
    gm = rm.max(axis=0)  # [nb] device global abs-max per image
    thr = gm * (1.0 - CAND_TOL)
    cb, ci = np.nonzero((rm >= thr[None, :]).T)  # image ids, candidate rows

    # exact scores for each candidate row, fp64
    row_idx = ci[:, None] + np.arange(KH)[None, :]  # [C, 16]
    strips = inputs_np[cb[:, None], row_idx, :]  # [C, 16, 128] f32
    win = np.lib.stride_tricks.sliding_window_view(strips, KW, axis=2)
    # win: [C, 16, 113, 16]
    wn64 = wn.astype(np.float64)
    n_cand = len(cb)
    scores = np.empty((n_cand, SW), dtype=np.float64)
    chunk = 8192
    for s in range(0, n_cand, chunk):
        e = min(s + chunk, n_cand)
        scores[s:e] = np.einsum(
            "cpjq,pq->cj", win[s:e].astype(np.float64), wn64, optimize=True
        )
    scores /= float(N_ELEM)

    # per image: among candidate rows pick max |score|, ties -> lowest flat idx
    flat = ci[:, None].astype(np.int64) * SW + np.arange(SW)[None, :]
    abss = np.abs(scores)
    # order candidates per image: sort by (image, -|score|, flat)
    best_val = np.zeros(nb, dtype=np.float64)
    best_flat = np.zeros(nb, dtype=np.int64)
    best_abs = np.full(nb, -1.0, dtype=np.float64)
    # reduce per candidate-row first
    j_best = np.argmax(abss, axis=1)  # first occurrence within row
    r_abs = abss[np.arange(n_cand), j_best]
    r_val = scores[np.arange(n_cand), j_best]
    r_flat = flat[np.arange(n_cand), j_best]
    # then reduce across rows of the same image (first occurrence on exact ties)
    order = np.lexsort((r_flat, -r_abs, cb))  # grouped by image
    cb_o = cb[order]
    first = np.unique(cb_o, return_index=True)[1]
    sel = order[first]
    img_ids = cb[sel]
    best_val[img_ids] = r_val[sel]
    best_flat[img_ids] = r_flat[sel]
    best_abs[img_ids] = r_abs[sel]
    assert np.all(best_abs >= 0.0), "some image had no candidate rows"

    rows = (best_flat // SW).astype(np.int64)
    cols = (best_flat % SW).astype(np.int64)
    vals = best_val.astype(np.float32)

    out = np.zeros((nb, H, W), dtype=np.float32)
    patches = vals[:, None, None] * wn[None, :, :]  # [nb, 16, 16] f32
    bidx = np.arange(nb)[:, None, None]
    ridx = rows[:, None, None] + np.arange(KH)[None, :, None]
    cidx = cols[:, None, None] + np.arange(KW)[None, None, :]
    out[bidx, ridx, cidx] = patches
    return out


def kernel(inputs: np.ndarray, w: np.ndarray) -> np.ndarray:
    inputs_np = np.ascontiguousarray(np.asarray(inputs, dtype=np.float32))
    wn = _weights_f32(w)
    ttoe = _toeplitz(wn)
    rm, _ = _run_device(inputs_np, ttoe)
    return _finalize(inputs_np, wn, rm)


# revision 9
# speedup vs baseline: 7.0282x; 7.0282x over previous
"""Trainium2 Bass kernel for GTPCA topk_masking layer.

Computation (see reference):
  wn     = w / sqrt(sum(w^2)/n),  n = 128*128
  scores = valid_xcorr2d(inputs, wn) / n          -> (B, 113, 113)
  idx    = argmax |scores| (flat, first occurrence)
  out    = scores[idx] * wn placed as a 16x16 patch at idx, zeros elsewhere

Device strategy (pure data parallel over 8 cores, 512 images each):
  - The 2D correlation is done on the tensor engine as 16 accumulating
    matmuls per image group: for each kernel column q, a 128x113 Toeplitz
    matrix T_q (T_q[h, i] = wn[h-i, q]) is the stationary operand and the
    width-shifted image slice in[:, j+q] is the moving operand, accumulating
    over q in PSUM.  Uses float32r so the PE streams 1 column/cycle.
  - Per PSUM bank (4 images) a single fused DVE reduce with
    apply_absolute_value produces per-row abs-max of the score map.
  - Only the per-row abs-max [113, 512] leaves the device.  The host finds
    candidate rows within 1% of each image's global max (fp32r noise is
    orders of magnitude smaller), rescores those rows exactly in fp64,
    picks the true argmax, and scatters smax*wn patches into the output.
"""

import sys

import numpy as np

if "/opt/trn_rl_repo" not in sys.path:
    sys.path.insert(0, "/opt/trn_rl_repo")

N_CORES = 8
B = 4096
H = W = 128
KH = KW = 16
SH = SW = H - KH + 1  # 113
SW_PAD = 114  # fp32r matmul needs even innermost moving/psum counts
W_PAD = 130  # padded image width so q+114 stays in bounds, stride even
N_ELEM = H * W  # 16384
PER_CORE = B // N_CORES  # 512
GROUP = 16  # images per DMA/compute group
BANK = 4  # images per PSUM bank (4*113 = 452 <= 512 moving-dim limit)
CAND_TOL = 1e-2  # candidate-row gate vs device global max


def _build_nc(n_imgs: int, repeat: int = 1):
    from contextlib import ExitStack

    import concourse.bacc as bacc
    import concourse.mybir as mybir
    import concourse.tile as tile

    f32 = mybir.dt.float32
    f32r = mybir.dt.float32r

    nc = bacc.Bacc("TRN2", target_bir_lowering=False)
    imgs_d = nc.dram_tensor("imgs", [H, n_imgs, W_PAD], f32r, kind="ExternalInput")
    ttoe_d = nc.dram_tensor("ttoe", [H, KW, SH], f32r, kind="ExternalInput")
    rm_d = nc.dram_tensor("rowmax", [SH, n_imgs], f32, kind="ExternalOutput")

    n_groups = n_imgs // GROUP
    banks_per_group = GROUP // BANK

    with ExitStack() as ctx:
        tc = ctx.enter_context(tile.TileContext(nc))
        consts = ctx.enter_context(tc.tile_pool(name="consts", bufs=1))
        imgp = ctx.enter_context(tc.tile_pool(name="imgp", bufs=3))
        accp = ctx.enter_context(tc.tile_pool(name="accp", bufs=2, space="PSUM"))
        stage = ctx.enter_context(tc.tile_pool(name="stage", bufs=1))

        ttoe_t = consts.tile([H, KW, SH], f32r)
        nc.sync.dma_start(ttoe_t[:], ttoe_d[:])
        rm_all = stage.tile([SH, n_imgs], f32)

        for _rep in range(repeat):
          for g in range(n_groups):
            img_t = imgp.tile([H, GROUP, W_PAD], f32r)
            nc.sync.dma_start(img_t[:], imgs_d[:, g * GROUP : (g + 1) * GROUP, :])

            psums = [
                accp.tile([SH, BANK, SW_PAD], f32, name=f"acc{bk}", tag=f"acc{bk}")
                for bk in range(banks_per_group)
            ]
            for q in range(KW):
                lhs = ttoe_t[:, q, :]
                for bk in range(banks_per_group):
                    rhs = img_t[:, bk * BANK : (bk + 1) * BANK, q : q + SW_PAD]
                    nc.tensor.matmul(
                        psums[bk][:],
                        lhs,
                        rhs,
                        start=(q == 0),
                        stop=(q == KW - 1),
                        skip_group_check=True,
                    )
            for bk in range(banks_per_group):
                base = g * GROUP + bk * BANK
                nc.vector.tensor_reduce(
                    rm_all[:, base : base + BANK],
                    psums[bk][:, :, 0:SW],
                    axis=mybir.AxisListType.X,
                    op=mybir.AluOpType.max,
                    apply_absolute_value=True,
                )

        nc.sync.dma_start(rm_d[:], rm_all[:])

    nc.compile()
    return nc


_NC_CACHE: dict = {}


def _get_nc(n_imgs: int):
    if n_imgs not in _NC_CACHE:
        _NC_CACHE[n_imgs] = _build_nc(n_imgs)
    return _NC_CACHE[n_imgs]


def _weights_f32(w: np.ndarray) -> np.ndarray:
    w32 = np.asarray(w, dtype=np.float32)
    ss = np.sum(w32 * w32, dtype=np.float32)
    denom = np.sqrt(ss / np.float32(N_ELEM))
    return (w32 / denom).astype(np.float32)


def _toeplitz(wn: np.ndarray) -> np.ndarray:
    ttoe = np.zeros((H, KW, SH), dtype=np.float32)
    for i in range(SH):
        ttoe[i : i + KH, :, i] = wn
    return ttoe


def _run_device(inputs_np: np.ndarray, ttoe: np.ndarray, trace: bool = False):
    from concourse.bass_utils import run_bass_kernel_spmd

    nc = _get_nc(PER_CORE)
    nb = inputs_np.shape[0]
    host_t = np.zeros((H, nb, W_PAD), dtype=np.float32)
    host_t[:, :, :W] = inputs_np.transpose(1, 0, 2)
    in_maps = []
    for c in range(N_CORES):
        shard = np.ascontiguousarray(
            host_t[:, c * PER_CORE : (c + 1) * PER_CORE, :]
        )
        in_maps.append({"imgs": shard, "ttoe": ttoe})
    res = run_bass_kernel_spmd(
        nc, in_maps, core_ids=list(range(N_CORES)), trace=trace
    )
    rm = np.concatenate([r["rowmax"] for r in res.results], axis=1)  # [113, B]
    return rm, res


def _finalize(inputs_np: np.ndarray, wn: np.ndarray, rm: np.ndarray) -> np.ndarray:
    """Host: candidate rows -> exact rescore -> argmax -> patch scatter."""
    nb = rm.shape[1]
    gm = rm.max(axis=0)  # [nb] device global abs-max per image
    thr = gm * (1.0 - CAND_TOL)
    cb, ci = np.nonzero((rm >= thr[None, :]).T)  # image ids, candidate rows

    # exact scores for each candidate row, fp64
    row_idx = ci[:, None] + np.arange(KH)[None, :]  # [C, 16]
    strips = inputs_np[cb[:, None], row_idx, :]  # [C, 16, 128] f32
    win = np.lib.stride_tricks.sliding_window_view(strips, KW, axis=2)
    # win: [C, 16, 113, 16]
    wn64 = wn.astype(np.float64)
    n_cand = len(cb)
    scores = np.empty((n_cand, SW), dtype=np.float64)
    chunk = 8192
    for s in range(0, n_cand, chunk):
        e = min(s + chunk, n_cand)
        scores[s:e] = np.einsum(
            "cpjq,pq->cj", win[s:e].astype(np.float64), wn64, optimize=True
        )
    scores /= float(N_ELEM)

    # per image: among candidate rows pick max |score|, ties -> lowest flat idx
    flat = ci[:, None].astype(np.int64) * SW + np.arange(SW)[None, :]
    abss = np.abs(scores)
    # order candidates per image: sort by (image, -|score|, flat)
    best_val = np.zeros(nb, dtype=np.float64)
    best_flat = np.zeros(nb, dtype=np.int64)
    best_abs = np.full(nb, -1.0, dtype=np.float64)
    # reduce per candidate-row first
    j_best = np.argmax(abss, axis=1)  # first occurrence within row
    r_abs = abss[np.arange(n_cand), j_best]
    r_val = scores[np.arange(n_cand), j_best]
    r_flat = flat[np.arange(n_cand), j_best]
    # then reduce across rows of the same image (first occurrence on exact ties)
    order = np.lexsort((r_flat, -r_abs, cb))  # grouped by image
    cb_o = cb[order]
    first = np.unique(cb_o, return_index=True)[1]
    sel = order[first]
    img_ids = cb[sel]
    best_val[img_ids] = r_val[sel]
    best_flat[img_ids] = r_flat[sel]
    best_abs[img_ids] = r_abs[sel]
    assert np.all(best_abs >= 0.0), "some image had no candidate rows"

    rows = (best_flat // SW).astype(np.int64)
    cols = (best_flat % SW).astype(np.int64)
    vals = best_val.astype(np.float32)

    out = np.zeros((nb, H, W), dtype=np.float32)
    patches = vals[:, None, None] * wn[None, :, :]  # [nb, 16, 16] f32
    bidx = np.arange(nb)[:, None, None]
    ridx = rows[:, None, None] + np.arange(KH)[None, :, None]
    cidx = cols[:, None, None] + np.arange(KW)[None, None, :]
    out[bidx, ridx, cidx] = patches
    return out


def kernel(inputs: np.ndarray, w: np.ndarray) -> np.ndarray:
    inputs_np = np.ascontiguousarray(np.asarray(inputs, dtype=np.float32))
    wn = _weights_f32(w)
    ttoe = _toeplitz(wn)
    rm, _ = _run_device(inputs_np, ttoe)
    return _finalize(inputs_np, wn, rm)
